# revision 1
# baseline (speedup 1.0000x reference)
"""Trainium2 Bass kernel for nn_Cross_Attention (2-batch, 16-head cross attention).

Sharding: 8 cores = 2 batches x 4 head-groups (4 heads each). Each core runs an
identical single-core Bass program on its (batch, head-group) slice; outputs are
disjoint column slices of the two full outputs, reassembled on the host.
"""

import math

import numpy as np

# Problem shapes (hardcoded per harness contract).
B = 2
N = 2048
DIMX = 1024
DIMY = 512
H = 16
D = 64
SCALE = 1.0 / 64.0
TOK_SCALE = 1.0 / math.sqrt(N)

NCORES = 8
GROUPS = NCORES // B          # 4 head-groups
HL = H // GROUPS              # 4 heads per core
HD = HL * D                   # 256 cols per core per tensor

P = 128
CX = DIMX // P                # 8 dim chunks of x
CY = DIMY // P                # 4 dim chunks of y
NT = N // P                   # 16 token tiles
NJ = N // 512                 # 4 token chunks of 512

_CACHE = {}


def _build():
    import concourse.bass as bass  # noqa: F401
    import concourse.mybir as mybir
    import concourse.tile as tile
    from concourse import bacc

    dt = mybir.dt
    f32, bf16, f32r = dt.float32, dt.bfloat16, dt.float32r
    EXP = mybir.ActivationFunctionType.Exp
    AX = mybir.AxisListType.X
    MAX = mybir.AluOpType.max

    nc = bacc.Bacc("TRN2", target_bir_lowering=False, debug=False, num_devices=NCORES)
    x = nc.dram_tensor("x", [N, DIMX], f32, kind="ExternalInput").ap()
    y = nc.dram_tensor("y", [N, DIMY], f32, kind="ExternalInput").ap()
    # wx packed [q(256) | v(256) | k(256)], wy packed [ks(256) | qs(256)]
    wx = nc.dram_tensor("wx", [DIMX, 3 * HD], f32, kind="ExternalInput").ap()
    wy = nc.dram_tensor("wy", [DIMY, 2 * HD], f32, kind="ExternalInput").ap()
    vout = nc.dram_tensor("vout", [N, HD], f32, kind="ExternalOutput").ap()
    oout = nc.dram_tensor("oout", [N, HD], f32, kind="ExternalOutput").ap()

    with tile.TileContext(nc) as tc:
        _emit(nc, tc, tile, mybir, x, y, wx, wy, vout, oout,
              f32=f32, bf16=bf16, f32r=f32r, EXP=EXP, AX=AX, MAX=MAX)
    nc.compile()
    return nc


def _emit(nc, tc, tile, mybir, x, y, wx, wy, vout, oout, *, f32, bf16, f32r, EXP, AX, MAX):
    ctxs = []

    def pool(name, bufs, space="SBUF"):
        p = tc.tile_pool(name=name, bufs=bufs, space=space)
        ctxs.append(p)
        return p.__enter__()

    wp = pool("wp", 1)
    dp = pool("dp", 1, "DRAM")     # bf16 staging in DRAM
    tp = pool("tp", 1)             # xT / yT persistent
    pp = pool("pp", 1)             # projection results persistent
    ep = pool("ep", 3)             # exp strips
    sm = pool("sm", 6)             # small per-(h,mi) tiles: denom/recip/vt
    t1p = pool("t1p", 4)           # t1 sbuf extraction
    cap = pool("cap", 1)           # chan_attn
    oop = pool("oop", 1)           # final out staging
    psd = pool("psd", 2, "PSUM")   # dots psum [128, 1024] x2 = 4 banks
    pst = pool("pst", 2, "PSUM")   # t1 psum [128, 512] x2 = 2 banks
    psp = pool("psp", 2, "PSUM")   # proj/chan/final psum [128, 512] x2 = 2 banks

    # ---- weights: HWDGE fp32 load (cheap descriptor gen), DVE cast to bf16.
    # wx is split so the k-columns (needed by kT early) land first.
    wy_f = wp.tile([P, CY, 2 * HD], f32)
    nc.sync.dma_start(wy_f[:], wy.rearrange("(c p) n -> p c n", p=P))
    wy_sb = wp.tile([P, CY, 2 * HD], bf16)
    nc.vector.tensor_copy(wy_sb[:], wy_f[:])
    wx_r = wx.rearrange("(c p) n -> p c n", p=P)
    wx_f = wp.tile([P, CX, 3 * HD], f32)
    wx_sb = wp.tile([P, CX, 3 * HD], bf16)

    def wx_load(lo, hi):
        nc.sync.dma_start(wx_f[:, :, lo:hi], wx_r[:, :, lo:hi])
        nc.vector.tensor_copy(wx_sb[:, :, lo:hi], wx_f[:, :, lo:hi])

    # warm the ACT exp table early
    warm = sm.tile([P, 1], f32, tag="warm")
    nc.vector.memset(warm[:], 0.0)
    warm2 = sm.tile([P, 1], f32, tag="warm2")
    nc.scalar.activation(warm2[:], warm[:], EXP)

    # ---- persistent on-chip tensors ----
    xT = tp.tile([P, CX, N], bf16)       # x^T: [dim%128, dim//128, n]
    yT = tp.tile([P, CY, N], bf16)
    qv_nat = pp.tile([P, NT, 2 * HD], bf16)   # [q | v] natural: [n%128, n//128, col]
    ks_nat = pp.tile([P, NT, HD], bf16)
    kT = pp.tile([P, 2, N], bf16)        # [kcol%128, kcol//128, n]
    qsT = pp.tile([P, 2, N], bf16)
    oo_sb = oop.tile([P, NT, HD], f32)
    vo_sb = oop.tile([P, NT, HD], f32, tag="vo")

    # ---- x/y ingest: flat contiguous SWDGE fp32->bf16 casts to DRAM
    # staging (cheap descriptor gen), then big-slab xbar transposes into
    # SBUF. Pipelined: y halves / x quarters.
    xb_dram = dp.tile([N, DIMX], bf16)
    yb_dram = dp.tile([N, DIMY], bf16)
    x_flat = x.rearrange("a b -> (a b)")
    y_flat = y.rearrange("a b -> (a b)")
    xb_flat = xb_dram.rearrange("a b -> (a b)")
    yb_flat = yb_dram.rearrange("a b -> (a b)")

    def x_cast(q):
        s = slice(q * 512 * DIMX, (q + 1) * 512 * DIMX)
        nc.gpsimd.dma_start(xb_flat[s], x_flat[s])

    def x_transpose(q):
        s = slice(q * 512, (q + 1) * 512)
        for c in range(CX):
            nc.sync.dma_start(xT[:, c, s], xb_dram[s, c * P:(c + 1) * P],
                              transpose=True)

    def y_cast(hh):
        s = slice(hh * 1024 * DIMY, (hh + 1) * 1024 * DIMY)
        nc.gpsimd.dma_start(yb_flat[s], y_flat[s])

    def y_transpose(hh):
        s = slice(hh * 1024, (hh + 1) * 1024)
        for c in range(CY):
            nc.sync.dma_start(yT[:, c, s], yb_dram[s, c * P:(c + 1) * P],
                              transpose=True)

    def qsT_chunk(m, j):
        acc = psp.tile([P, 512], f32, tag="proj")
        for c in range(CY):
            nc.tensor.matmul(acc[:], wy_sb[:, c, HD + m * P:HD + (m + 1) * P],
                             yT[:, c, j * 512:(j + 1) * 512],
                             start=(c == 0), stop=(c == CY - 1))
        nc.vector.tensor_copy(qsT[:, m, j * 512:(j + 1) * 512], acc[:])

    def kT_chunk(m, j):
        acc = psp.tile([P, 512], f32, tag="proj")
        for c in range(CX):
            nc.tensor.matmul(acc[:], wx_sb[:, c, 2 * HD + m * P:2 * HD + (m + 1) * P],
                             xT[:, c, j * 512:(j + 1) * 512],
                             start=(c == 0), stop=(c == CX - 1))
        nc.vector.tensor_copy(kT[:, m, j * 512:(j + 1) * 512], acc[:])

    # ---- per-(head, token-chunk) extra work slots, interleaved into head windows ----
    def qv_proj(j):
        acc = psp.tile([P, 512], f32, tag="proj")
        for c in range(CX):
            nc.tensor.matmul(acc[:], xT[:, c, j * P:(j + 1) * P], wx_sb[:, c, 0:512],
                             start=(c == 0), stop=(c == CX - 1))
        nc.vector.tensor_copy(qv_nat[:, j, :], acc[:])
        nc.vector.tensor_copy(vo_sb[:, j, :], acc[:, HD:2 * HD])

    def ks_proj(j):
        acc = psp.tile([P, 512], f32, tag="proj")
        for c in range(CY):
            nc.tensor.matmul(acc[:, 0:HD], yT[:, c, j * P:(j + 1) * P], wy_sb[:, c, 0:HD],
                             start=(c == 0), stop=(c == CY - 1))
        nc.vector.tensor_copy(ks_nat[:, j, :], acc[:, 0:HD])

    # prologue: y half 0 + x quarter 0 casts first; qsT/kT/qv for the early
    # region; the rest of the ingest streams in under head-0's window.
    y_cast(0)
    wx_load(2 * HD, 3 * HD)         # k columns (kT needs them early)
    x_cast(0)
    y_transpose(0)
    qsT_chunk(0, 0)
    qsT_chunk(0, 1)
    y_cast(1)
    x_cast(1)
    x_transpose(0)
    kT_chunk(0, 0)
    y_transpose(1)
    qsT_chunk(0, 2)
    qsT_chunk(0, 3)
    x_cast(2)
    x_cast(3)
    wx_load(0, 2 * HD)              # q|v columns
    for j in range(4):
        qv_proj(j)

    vq = vout.rearrange("(j p) c -> p j c", p=P)

    def extra_work(h, mi):
        if h == 0:
            if mi in (0, 4, 8):
                x_transpose(mi // 4 + 1)
            elif mi in (1, 5, 9):
                kT_chunk(0, mi // 4 + 1)
            if 3 <= mi <= 14:
                qv_proj(mi + 1)
            if mi == 15:
                for g in range(NJ):
                    nc.sync.dma_start(vq[:, 4 * g:4 * g + 4, :],
                                      vo_sb[:, 4 * g:4 * g + 4, :])
        elif h == 1:
            if mi % 4 == 0:
                kT_chunk(1, mi // 4)
            elif mi % 4 == 2:
                qsT_chunk(1, mi // 4)
        elif h == 2:
            ks_proj(mi)

    # ---- chan attention (per head, duplicated into both partition halves) ----
    ca_sb = cap.tile([P, HL, D], f32r)

    def chan_attn(h):
        cd = psp.tile([P, 512], f32, tag="proj")
        for half in range(2):
            for j in range(NT):
                nc.tensor.matmul(cd[64 * half:64 * half + 64, 0:D],
                                 qv_nat[:, j, h * D:(h + 1) * D],
                                 ks_nat[:, j, h * D:(h + 1) * D],
                                 start=(j == 0), stop=(j == NT - 1),
                                 tile_position=(0, 64 * half))
        mx = sm.tile([P, 1], f32, tag="cmx")
        nc.vector.tensor_reduce(mx[:], cd[:, 0:D], axis=AX, op=MAX, negate=True)
        mxs = sm.tile([P, 1], f32, tag="cms")
        nc.vector.tensor_scalar_mul(mxs[:], mx[:], SCALE)
        ce = sm.tile([P, D], f32, tag="ce")
        csum = sm.tile([P, 1], f32, tag="csum")
        nc.scalar.activation(ce[:], cd[:, 0:D], EXP, scale=SCALE, bias=mxs[:],
                             accum_out=csum[:])
        crec = sm.tile([P, 1], f32, tag="crec")
        nc.vector.reciprocal(crec[:], csum[:])
        nc.vector.tensor_scalar_mul(ca_sb[:, h, :], ce[:], crec[:])

    # ---- token attention + t1 per head ----
    def head_window(h):
        mchunk = h // 2
        base = 64 * (h % 2)
        t1ps = [pst.tile([P, 512], f32, tag="t1", name=f"t1ps{t}") for t in range(2)]
        for t in range(2):
            nc.vector.memset(t1ps[t][:], 0.0)

        def dots_exp(mi):
            e = ep.tile([P, N], bf16, tag="e", name="e")
            den2 = sm.tile([P, 2], f32, tag="den2", name="den2")
            for half in range(2):
                dots = psd.tile([P, N // 2], f32, tag="dots", name=f"dots{half}")
                for j in (0, 1):
                    jj = 2 * half + j
                    nc.tensor.matmul(dots[:, j * 512:(j + 1) * 512],
                                     kT[base:base + 64, mchunk, mi * P:(mi + 1) * P],
                                     qsT[base:base + 64, mchunk,
                                         jj * 512:(jj + 1) * 512],
                                     start=True, stop=True)
                nc.scalar.activation(e[:, half * 1024:(half + 1) * 1024], dots[:],
                                     EXP, scale=TOK_SCALE,
                                     accum_out=den2[:, half:half + 1])
            return e, den2

        pend = dots_exp(0)
        for mi in range(NT):
            e, den2 = pend
            if mi + 1 < NT:
                pend = dots_exp(mi + 1)
            extra_work(h, mi)
            rec = sm.tile([P, 1], f32, tag="rec")
            den = sm.tile([P, 1], f32, tag="den")
            nc.vector.reduce_sum(den[:], den2[:], axis=AX)
            nc.vector.reciprocal(rec[:], den[:])
            vt = sm.tile([P, D], bf16, tag="vt")
            nc.vector.tensor_scalar_mul(vt[:], qv_nat[:, mi, HD + h * D:HD + (h + 1) * D],
                                        rec[:])
            for j in range(NJ):
                nc.tensor.matmul(t1ps[j // 2][64 * (j % 2):64 * (j % 2) + 64, :],
                                 vt[:],
                                 e[:, j * 512:(j + 1) * 512],
                                 start=False, stop=False,
                                 tile_position=(0, 64 * (j % 2)),
                                 skip_group_check=True)
        t1sb = t1p.tile([P, 2, 512], f32r, tag="t1sb")
        for t in range(2):
            nc.vector.tensor_copy(t1sb[:, t, :], t1ps[t][:])
        return t1sb

    def final_out(h, t1sb):
        for g in range(NT // 4):
            acc = psp.tile([P, 512], f32, tag="proj")
            accv = acc.rearrange("p (j d) -> p j d", d=D)
            nc.vector.memset(acc[:, 0:4 * D], 0.0)
            for sj in range(4):
                j = 4 * g + sj
                jj = j // 4
                bb = 64 * (jj % 2)
                nc.tensor.matmul(accv[:, sj, :],
                                 t1sb[bb:bb + 64, jj // 2,
                                      (j % 4) * P:(j % 4 + 1) * P],
                                 ca_sb[bb:bb + 64, h, :],
                                 start=False, stop=False,
                                 skip_group_check=True)
            nc.vector.tensor_copy(
                oo_sb[:, 4 * g:4 * g + 4, h * D:(h + 1) * D],
                accv[:, 0:4, 0:D])

    t1_saved = {}
    for h in range(3):
        t1_saved[h] = head_window(h)
    for h in range(3):
        chan_attn(h)
        final_out(h, t1_saved[h])
    t1_saved[3] = head_window(3)
    chan_attn(3)
    final_out(3, t1_saved[3])

    oor = oout.rearrange("(j p) c -> p j c", p=P)
    for g in range(NJ):
        nc.sync.dma_start(oor[:, 4 * g:4 * g + 4, :], oo_sb[:, 4 * g:4 * g + 4, :])

    for p in reversed(ctxs):
        p.__exit__(None, None, None)


def _get_prog():
    if "nc" not in _CACHE:
        _CACHE["nc"] = _build()
    return _CACHE["nc"]


def kernel(x, y, W_qkv, W_qkv_side):
    from concourse.bass_utils import run_bass_kernel_spmd

    nc = _get_prog()
    x = np.asarray(x, dtype=np.float32)
    y = np.asarray(y, dtype=np.float32)
    W_qkv = np.asarray(W_qkv, dtype=np.float32)
    W_qkv_side = np.asarray(W_qkv_side, dtype=np.float32)

    in_maps = []
    for c in range(NCORES):
        b, g = divmod(c, GROUPS)
        lo, hi = g * HD, (g + 1) * HD
        wx_c = np.ascontiguousarray(np.concatenate(
            [W_qkv[:, lo:hi], W_qkv[:, 2 * DIMX + lo:2 * DIMX + hi],
             W_qkv[:, DIMX + lo:DIMX + hi]], axis=1))
        wy_c = np.ascontiguousarray(np.concatenate(
            [W_qkv_side[:, DIMX + lo:DIMX + hi], W_qkv_side[:, lo:hi]], axis=1))
        in_maps.append({
            "x": np.ascontiguousarray(x[b]),
            "y": np.ascontiguousarray(y[b]),
            "wx": wx_c,
            "wy": wy_c,
        })

    _CACHE["in_maps_last"] = in_maps
    res = run_bass_kernel_spmd(nc, in_maps, core_ids=list(range(NCORES)))
    _CACHE["last_results"] = res

    v_full = np.empty((B, N, H * D), dtype=np.float32)
    o_full = np.empty((B, N, H * D), dtype=np.float32)
    for c in range(NCORES):
        b, g = divmod(c, GROUPS)
        v_full[b, :, g * HD:(g + 1) * HD] = res.results[c]["vout"]
        o_full[b, :, g * HD:(g + 1) * HD] = res.results[c]["oout"]
    return (v_full, o_full)



# revision 26
# speedup vs baseline: 1.1975x; 1.1975x over previous
"""Trainium2 Bass kernel for nn_Cross_Attention (2-batch, 16-head cross attention).

Sharding: 8 cores = 2 batches x 4 head-groups (4 heads each). Each core runs an
identical single-core Bass program on its (batch, head-group) slice; outputs are
disjoint column slices of the two full outputs, reassembled on the host.

Structure per core (heads h=0..3, token tiles mi=0..15):
  ingest:       x/y cast fp32->bf16 straight into SBUF natural layout (SWDGE),
                transposed on-chip via PE-mode transposes (x q2/q3 via DRAM
                staging + xbar DMA transpose to spread load).
  projections:  q/v (natural), kT, qsT (transposed), ks (natural)
  per (h, mi):  dots[m,n] = k_mi . qs  ->  e = exp(dots * s) (ACT, row-sum accum)
                vt = v_mi / Z          ->  ot[n,d] += e[:,ntile]^T @ vt
  per head:     ot -> sbuf -> PE-transpose -> t1[d,n];  chan attn ca[d,e];
                out[n,e] = t1_slice @ ca per token tile.
"""

import math

import numpy as np

# Problem shapes (hardcoded per harness contract).
B = 2
N = 2048
DIMX = 1024
DIMY = 512
H = 16
D = 64
SCALE = 1.0 / 64.0
TOK_SCALE = 1.0 / math.sqrt(N)

NCORES = 8
GROUPS = NCORES // B          # 4 head-groups
HL = H // GROUPS              # 4 heads per core
HD = HL * D                   # 256 cols per core per tensor

P = 128
CX = DIMX // P                # 8 dim chunks of x
CY = DIMY // P                # 4 dim chunks of y
NT = N // P                   # 16 token tiles
NJ = N // 512                 # 4 token chunks of 512

_CACHE = {}


def _build():
    import concourse.bass as bass  # noqa: F401
    import concourse.mybir as mybir
    import concourse.tile as tile
    from concourse import bacc

    dt = mybir.dt
    f32, bf16 = dt.float32, dt.bfloat16
    EXP = mybir.ActivationFunctionType.Exp
    AX = mybir.AxisListType.X
    MAX = mybir.AluOpType.max

    nc = bacc.Bacc("TRN2", target_bir_lowering=False, debug=False, num_devices=NCORES)
    x = nc.dram_tensor("x", [N, DIMX], f32, kind="ExternalInput").ap()
    y = nc.dram_tensor("y", [N, DIMY], f32, kind="ExternalInput").ap()
    # wx packed [q(256) | v(256) | k(256)], wy packed [ks(256) | qs(256)]
    wx = nc.dram_tensor("wx", [DIMX, 3 * HD], f32, kind="ExternalInput").ap()
    wy = nc.dram_tensor("wy", [DIMY, 2 * HD], f32, kind="ExternalInput").ap()
    vout = nc.dram_tensor("vout", [N, HD], f32, kind="ExternalOutput").ap()
    oout = nc.dram_tensor("oout", [N, HD], f32, kind="ExternalOutput").ap()

    with tile.TileContext(nc) as tc:
        _emit(nc, tc, tile, mybir, x, y, wx, wy, vout, oout,
              f32=f32, bf16=bf16, EXP=EXP, AX=AX, MAX=MAX)
    nc.compile()
    return nc


def _emit(nc, tc, tile, mybir, x, y, wx, wy, vout, oout, *, f32, bf16, EXP, AX, MAX):
    from concourse.masks import make_identity

    ctxs = []

    def pool(name, bufs, space="SBUF"):
        p = tc.tile_pool(name=name, bufs=bufs, space=space)
        ctxs.append(p)
        return p.__enter__()

    wp = pool("wp", 1)
    dp = pool("dp", 1, "DRAM")     # bf16 staging for x q1-3 DMA transposes
    tp = pool("tp", 1)             # xT / yT persistent
    xnp = pool("xnp", 2)           # x natural quarters (transient)
    ynp = pool("ynp", 4)           # y natural quarters (transient)
    pp = pool("pp", 1)             # projection results persistent
    ep = pool("ep", 6)             # exp strips (deep for lagged window 0)
    sm = pool("sm", 6)             # small per-(h,mi) tiles: denom/recip/vt
    otp = pool("otp", 1)           # ot sbuf staging per head
    t1p = pool("t1p", 1)           # t1 (transposed ot) persistent
    cap = pool("cap", 1)           # chan_attn
    oop = pool("oop", 1)           # final out staging
    psd = pool("psd", 2, "PSUM")   # dots psum [128, 1024] x2 = 4 banks
    pso = pool("pso", 1, "PSUM")   # ot psum [128, 1024] = 2 banks
    sc = pool("sc", 2, "PSUM")     # scratch psum [128, 512] x2 = 2 banks

    # ---- weights: SWDGE fp32->bf16 strided casts straight into SBUF. The
    # Pool descriptor-gen queue serializes them behind the x/y ingest casts,
    # which is exactly the arrival order we want (k/qs first, then v/q/ks).
    wy_sb = wp.tile([P, CY, 2 * HD], bf16)
    wy_r = wy.rearrange("(c p) n -> p c n", p=P)
    wx_sb = wp.tile([P, CX, 3 * HD], bf16)
    wx_r = wx.rearrange("(c p) n -> p c n", p=P)

    def wx_load_swdge(lo, hi):
        nc.gpsimd.dma_start(wx_sb[:, :, lo:hi], wx_r[:, :, lo:hi])

    def wy_load_swdge(lo, hi):
        nc.gpsimd.dma_start(wy_sb[:, :, lo:hi], wy_r[:, :, lo:hi])

    # warm the ACT exp table early
    warm = sm.tile([P, 1], f32, tag="warm")
    nc.vector.memset(warm[:], 0.0)
    warm2 = sm.tile([P, 1], f32, tag="warm2")
    nc.scalar.activation(warm2[:], warm[:], EXP)

    # identity for PE-mode transposes (built after cast descriptor-gens are
    # queued so it does not delay them on the Pool engine; see prologue)
    ident = wp.tile([P, P], bf16)

    # ---- persistent on-chip tensors ----
    xT = tp.tile([P, CX, N], bf16)       # x^T: [dim%128, dim//128, n]
    yT = tp.tile([P, CY, N], bf16)
    q_nat = pp.tile([P, NT, HD], bf16)   # q natural: [n%128, n//128, col]
    ks_nat = pp.tile([P, NT, HD], bf16)
    kT = pp.tile([P, 2, N], bf16)        # [kcol%128, kcol//128, n]
    qsT = pp.tile([P, 2, N], bf16)
    t1sb = t1p.tile([P, 2, N], bf16)     # t1[d, n]: head h at part 64*(h%2), slot h//2
    ca_sb = cap.tile([P, 2, D], bf16)    # chan attn: head h at part 64*(h%2), slot h//2
    vo_sb = oop.tile([P, NT, HD], f32, tag="vo")

    # ---- x/y ingest ----
    x_r = x.rearrange("(j p) c -> p j c", p=P)    # [128, 16, 1024] f32
    y_r = y.rearrange("(j p) c -> p j c", p=P)    # [128, 16, 512] f32
    x_flat = x.rearrange("a b -> (a b)")
    xb_dram = dp.tile([512, DIMX], bf16)          # ONE quarter: WAR deps chain
    xb_flat = xb_dram.rearrange("a b -> (a b)")

    def x_cast(q):
        xn = xnp.tile([P, 4, DIMX], bf16, tag="xn", name=f"xn{q}")
        nc.gpsimd.dma_start(xn[:], x_r[:, 4 * q:4 * q + 4, :])
        return xn

    def x_cast_dram(q):
        # flat fp32->bf16 cast of x quarter q into the shared staging slab;
        # WAW against the previous quarter's transposes sequences the chain.
        nc.gpsimd.dma_start(
            xb_flat[:], x_flat[q * 512 * DIMX:(q + 1) * 512 * DIMX])

    def x_dma_tr(q):
        for c in range(CX):
            nc.sync.dma_start(xT[:, c, q * 512:(q + 1) * 512],
                              xb_dram[:, c * P:(c + 1) * P], transpose=True)

    def y_cast(q):
        yn = ynp.tile([P, 4, DIMY], bf16, tag="yn", name=f"yn{q}")
        nc.gpsimd.dma_start(yn[:], y_r[:, 4 * q:4 * q + 4, :])
        return yn

    def _pe_tr(q, src_nat, dst, c0, c1, scalar_copy=False):
        # transpose chunks [c0, c1) of quarter q into dst, 2 chunks per psum
        # tile; pre-ladder copies can ride the idle Scalar engine.
        for c in range(c0, c1, 2):
            tT = sc.tile([P, 1024], bf16, tag="sc")
            for cc in range(2):
                for j in range(4):
                    nc.tensor.transpose(
                        tT[:, cc * 512 + j * P:cc * 512 + (j + 1) * P],
                        src_nat[:, j, (c + cc) * P:(c + cc + 1) * P], ident[:])
            tv = tT.rearrange("p (cc n) -> p cc n", cc=2)
            if scalar_copy:
                nc.scalar.copy(dst[:, c:c + 2, q * 512:(q + 1) * 512], tv[:])
            else:
                nc.vector.tensor_copy(dst[:, c:c + 2, q * 512:(q + 1) * 512], tv[:])

    def x_pe_tr(q, xn, c0, c1, scalar_copy=False):
        _pe_tr(q, xn, xT, c0, c1, scalar_copy)

    def y_pe_tr(q, yn, c0, c1, scalar_copy=False):
        _pe_tr(q, yn, yT, c0, c1, scalar_copy)

    def qsT_chunk(m, j):
        acc = sc.tile([P, 512], f32, tag="sc")
        for c in range(CY):
            nc.tensor.matmul(acc[:], wy_sb[:, c, HD + m * P:HD + (m + 1) * P],
                             yT[:, c, j * 512:(j + 1) * 512],
                             start=(c == 0), stop=(c == CY - 1))
        nc.vector.tensor_copy(qsT[:, m, j * 512:(j + 1) * 512], acc[:])

    def kT_chunk(m, j):
        acc = sc.tile([P, 512], f32, tag="sc")
        for c in range(CX):
            nc.tensor.matmul(acc[:], wx_sb[:, c, 2 * HD + m * P:2 * HD + (m + 1) * P],
                             xT[:, c, j * 512:(j + 1) * 512],
                             start=(c == 0), stop=(c == CX - 1))
        nc.vector.tensor_copy(kT[:, m, j * 512:(j + 1) * 512], acc[:])

    # ---- per-token-tile projections (interleaved into head windows) ----
    def v_proj(j):
        acc = sc.tile([P, 512], f32, tag="sc")
        for c in range(CX):
            nc.tensor.matmul(acc[:, 0:HD], xT[:, c, j * P:(j + 1) * P],
                             wx_sb[:, c, HD:2 * HD],
                             start=(c == 0), stop=(c == CX - 1))
        nc.vector.tensor_copy(vo_sb[:, j, :], acc[:, 0:HD])

    def q_proj(j, lo=0, hi=HD):
        acc = sc.tile([P, 512], f32, tag="sc")
        for c in range(CX):
            nc.tensor.matmul(acc[:, 0:hi - lo], xT[:, c, j * P:(j + 1) * P],
                             wx_sb[:, c, lo:hi],
                             start=(c == 0), stop=(c == CX - 1))
        nc.vector.tensor_copy(q_nat[:, j, lo:hi], acc[:, 0:hi - lo])

    def ks_proj(j, lo=0, hi=HD):
        acc = sc.tile([P, 512], f32, tag="sc")
        for c in range(CY):
            nc.tensor.matmul(acc[:, 0:hi - lo], yT[:, c, j * P:(j + 1) * P],
                             wy_sb[:, c, lo:hi],
                             start=(c == 0), stop=(c == CY - 1))
        nc.vector.tensor_copy(ks_nat[:, j, lo:hi], acc[:, 0:hi - lo])

    # ---- chan attention (head h lives on partitions bb:bb+64), split into
    # a dots stage and a softmax stage so the ACT exp never waits on the PE
    # accumulation chain inside one extra slot.
    chan_cd = {}

    def chan_dots(h):
        bb = 64 * (h % 2)
        cd = sc.tile([P, 512], f32, tag="sc")
        for j in range(NT):
            nc.tensor.matmul(cd[bb:bb + 64, 0:D],
                             q_nat[:, j, h * D:(h + 1) * D],
                             ks_nat[:, j, h * D:(h + 1) * D],
                             start=(j == 0), stop=(j == NT - 1),
                             tile_position=(0, bb))
        chan_cd[h] = cd

    def chan_soft(h):
        bb = 64 * (h % 2)
        cd = chan_cd[h]
        mx = sm.tile([P, 1], f32, tag="cmx")
        nc.vector.tensor_reduce(mx[bb:bb + 64], cd[bb:bb + 64, 0:D], axis=AX,
                                op=MAX, negate=True)
        mxs = sm.tile([P, 1], f32, tag="cms")
        nc.vector.tensor_scalar_mul(mxs[bb:bb + 64], mx[bb:bb + 64], SCALE)
        ce = sm.tile([P, D], f32, tag="ce")
        csum = sm.tile([P, 1], f32, tag="csum")
        nc.scalar.activation(ce[bb:bb + 64], cd[bb:bb + 64, 0:D], EXP, scale=SCALE,
                             bias=mxs[bb:bb + 64], accum_out=csum[bb:bb + 64])
        crec = sm.tile([P, 1], f32, tag="crec")
        nc.vector.reciprocal(crec[bb:bb + 64], csum[bb:bb + 64])
        nc.vector.tensor_scalar_mul(ca_sb[bb:bb + 64, h // 2, :], ce[bb:bb + 64],
                                    crec[bb:bb + 64])

    # ---- unified stream: 64 (h, mi) iterations, global lag-4 ot flush ----
    otps = {}
    _osb = {}

    def dots_half(h, mi, half, e, den2):
        mchunk = h // 2
        base = 64 * (h % 2)
        dots = psd.tile([P, N // 2], f32, tag="dots", name=f"dots{half}")
        for j in (0, 1):
            jj = 2 * half + j
            nc.tensor.matmul(dots[:, j * 512:(j + 1) * 512],
                             kT[base:base + 64, mchunk, mi * P:(mi + 1) * P],
                             qsT[base:base + 64, mchunk, jj * 512:(jj + 1) * 512],
                             start=True, stop=True)
        nc.scalar.activation(e[:, half * 1024:(half + 1) * 1024], dots[:],
                             EXP, scale=TOK_SCALE,
                             accum_out=den2[:, half:half + 1])

    def make_item(h, mi):
        e = ep.tile([P, N], bf16, tag="e", name="e")
        den2 = sm.tile([P, 2], f32, tag="den2", name="den2")
        return (h, mi, e, den2)

    def dots_exp(h, mi):
        it = make_item(h, mi)
        dots_half(h, mi, 0, it[2], it[3])
        dots_half(h, mi, 1, it[2], it[3])
        return it

    def ot_stage(h):
        osb = otp.tile([P, NT * D], bf16, tag="osb")
        nc.vector.tensor_copy(osb[:], otps[h][:])
        return osb

    def ot_flush(item):
        h, mi, e, den2 = item
        if mi == 0:
            otps[h] = pso.tile([P, NT * D], f32, tag="ot", name=f"otps{h}")
            nc.vector.memset(otps[h][:], 0.0)
        rec = sm.tile([P, 1], f32, tag="rec")
        den = sm.tile([P, 1], f32, tag="den")
        nc.vector.tensor_add(den[:], den2[:, 0:1], den2[:, 1:2])
        nc.vector.reciprocal(rec[:], den[:])
        vt = sm.tile([P, D], bf16, tag="vt")
        if h < 3:
            nc.vector.tensor_scalar_mul(vt[:], vo_sb[:, mi, h * D:(h + 1) * D],
                                        rec[:])
        else:
            # head 3: chan mixing pre-applied (cox = x @ (Wv @ ca3)), so the
            # ot accumulation directly produces the final output rows.
            nc.vector.tensor_scalar_mul(vt[:], cox_sb[:, mi, :], rec[:])
        for nt in range(NT):
            nc.tensor.matmul(otps[h][:, nt * D:(nt + 1) * D],
                             e[:, nt * P:(nt + 1) * P],
                             vt[:],
                             start=False, stop=False,
                             skip_group_check=True)
        if mi == NT - 1 and h < 3:
            _osb[h] = ot_stage(h)

    # ---- post-window head work: ot -> sbuf -> transpose -> t1sb ----
    def transpose_group(h, g):
        bb = 64 * (h % 2)
        osb = _osb[h]
        tT = sc.tile([P, 512], bf16, tag="sc")
        for s in range(4):
            nt = 4 * g + s
            nc.tensor.transpose(tT[bb:bb + 64, s * P:(s + 1) * P],
                                osb[:, nt * D:(nt + 1) * D],
                                ident[:],
                                tile_position=(0, bb))
        nc.vector.tensor_copy(t1sb[bb:bb + 64, h // 2, g * 512:(g + 1) * 512],
                              tT[bb:bb + 64, :])

    oor = oout.rearrange("(j p) c -> p j c", p=P)

    def final_half(h, g2):
        bb = 64 * (h % 2)
        acc = sc.tile([P, 512], f32, tag="sc")
        for s in range(8):
            nt = 8 * g2 + s
            nc.tensor.matmul(acc[:, s * D:(s + 1) * D],
                             t1sb[bb:bb + 64, h // 2, nt * P:(nt + 1) * P],
                             ca_sb[bb:bb + 64, h // 2, :],
                             start=True, stop=True,
                             skip_group_check=True)
        oos = oop.tile([P, 512], f32, tag="oo", bufs=2)
        nc.vector.tensor_copy(oos[:], acc[:])
        oosv = oos.rearrange("p (s d) -> p s d", d=D)
        nc.sync.dma_start(oor[:, 8 * g2:8 * g2 + 8, h * D:(h + 1) * D],
                          oosv[:])

    def final_out(h):
        final_half(h, 0)
        final_half(h, 1)

    # ---- head-3 pre-mixed value projection: cox = x @ (Wv_3 @ ca_3) ----
    wvT = cap.tile([P, CX, P], bf16, tag="wvT")       # parts 64:128 used
    wvca = cap.tile([P, CX, D], bf16, tag="wvca")
    cox_sb = cap.tile([P, NT, D], bf16, tag="cox")

    def wvca_stage():
        # transpose Wv (head-3 columns) then multiply by ca_3
        for g in range(2):
            tT = sc.tile([P, 1024], bf16, tag="sc")
            for cc in range(4):
                c = 4 * g + cc
                nc.tensor.transpose(tT[64:128, cc * P:(cc + 1) * P],
                                    wx_sb[:, c, HD + 3 * D:HD + 4 * D],
                                    ident[:], tile_position=(0, 64))
            tv = tT[:, 0:512].rearrange("p (cc n) -> p cc n", cc=4)
            nc.vector.tensor_copy(wvT[64:128, 4 * g:4 * g + 4, :], tv[64:128])
        acc = sc.tile([P, 512], f32, tag="sc")
        av = acc.rearrange("p (c d) -> p c d", d=D)
        for c in range(CX):
            nc.tensor.matmul(av[:, c, :], wvT[64:128, c, :],
                             ca_sb[64:128, 1, :],
                             start=True, stop=True, skip_group_check=True)
        nc.vector.tensor_copy(wvca[:], av[:])

    def cox_proj(j):
        acc = sc.tile([P, 512], f32, tag="sc")
        for c in range(CX):
            nc.tensor.matmul(acc[:, 0:D], xT[:, c, j * P:(j + 1) * P],
                             wvca[:, c, :],
                             start=(c == 0), stop=(c == CX - 1))
        nc.vector.tensor_copy(cox_sb[:, j, :], acc[:, 0:D])

    # ---- prologue ----
    # Critical DMA chain: x q0 cast -> PE transpose -> kT(0,0);
    # wx(k)/wy(qs) SWDGE casts; y quarters -> qsT(0, j).
    xn0 = x_cast(0)
    wx_load_swdge(2 * HD, 3 * HD)   # k columns (both m-chunks)
    wy_load_swdge(HD, 2 * HD)       # qs columns
    make_identity(nc, ident[:])
    yn0 = y_cast(0)
    yn1 = y_cast(1)
    yn2 = y_cast(2)
    yn3 = y_cast(3)
    xn1 = x_cast(1)
    xn2 = x_cast(2)
    wx_load_swdge(HD, 2 * HD)       # v columns
    x_pe_tr(0, xn0, 0, CX, scalar_copy=True)
    kT_chunk(0, 0)
    y_pe_tr(0, yn0, 0, CY, scalar_copy=True)
    qsT_chunk(0, 0)
    y_pe_tr(1, yn1, 0, CY, scalar_copy=True)
    qsT_chunk(0, 1)

    # ladder: first three mi of head 0, half A only (half B waits on y q2/q3)
    items = []
    for mi in range(3):
        items.append(make_item(0, mi))
        dots_half(0, mi, 0, items[mi][2], items[mi][3])
    y_pe_tr(2, yn2, 0, CY)
    qsT_chunk(0, 2)
    y_pe_tr(3, yn3, 0, CY)
    qsT_chunk(0, 3)
    for mi in range(3):
        dots_half(0, mi, 1, items[mi][2], items[mi][3])
    x_pe_tr(1, xn1, 0, CX)
    kT_chunk(0, 1)
    v_proj(0)

    vq = vout.rearrange("(j p) c -> p j c", p=P)
    xns = {2: xn2}
    H3 = 3 * D  # head-3 column offset

    def extra_h0(mi):
        if mi == 3:
            x_pe_tr(2, xns[2], 0, CX)
            xns[3] = x_cast(3)
        elif mi == 5:
            kT_chunk(0, 2)
        elif mi == 6:
            x_pe_tr(3, xns[3], 0, 4)
            wx_load_swdge(0, HD)            # q columns
        elif mi == 7:
            x_pe_tr(3, xns[3], 4, 8)
        elif mi == 9:
            kT_chunk(0, 3)
        elif mi == 11:
            wy_load_swdge(0, HD)            # ks columns
        if 3 <= mi <= 15:
            v_proj(mi - 2)
        if mi == 15:
            for g in range(NJ - 1):
                nc.sync.dma_start(vq[:, 4 * g:4 * g + 4, :],
                                  vo_sb[:, 4 * g:4 * g + 4, :])

    def extra_h1(mi):
        if mi % 4 == 0:
            kT_chunk(1, mi // 4)
        elif mi == 1:
            v_proj(14)
            for j in range(0, 4):
                q_proj(j, H3, HD)
        elif mi % 4 == 2:
            qsT_chunk(1, mi // 4)
            if mi == 2:
                v_proj(15)
        elif mi == 3:
            for j in range(4, 8):
                q_proj(j, H3, HD)
            nc.sync.dma_start(vq[:, 12:16, :], vo_sb[:, 12:16, :])
        elif mi == 9:
            for j in range(8, 12):
                q_proj(j, H3, HD)
        elif mi == 13:
            for j in range(12, 16):
                q_proj(j, H3, HD)
            for j in range(0, 4):
                ks_proj(j, H3, HD)
        else:  # mi in (5, 7, 11, 15)
            transpose_group(0, (mi - 3) // 4)

    def extra_h2(mi):
        ks_proj(mi, 0, H3)
        if mi == 0:
            for j in range(4, 10):
                ks_proj(j, H3, HD)
        elif mi == 1:
            for j in range(10, 16):
                ks_proj(j, H3, HD)
            q_proj(0, 0, H3)
        elif mi in (4, 6, 10, 14):
            transpose_group(1, (4, 6, 10, 14).index(mi))
            if mi == 14:
                q_proj(7, 0, H3)
        elif mi % 2 == 1 and mi < 13:
            q_proj((mi - 1) // 2, 0, H3)
        elif mi == 13:
            chan_dots(3)
            q_proj(6, 0, H3)
        elif mi == 15:
            chan_soft(3)
            wvca_stage()

    def extra_h3(mi):
        if mi < 8:
            q_proj(8 + mi, 0, H3)
            cox_proj(2 * mi)
            cox_proj(2 * mi + 1)
            if 2 <= mi < 6:
                transpose_group(2, mi - 2)
        elif mi == 8:
            chan_dots(0)
        elif mi == 9:
            chan_dots(1)
            chan_soft(0)
        elif mi == 10:
            chan_dots(2)
            chan_soft(1)
            final_out(0)
        elif mi == 11:
            chan_soft(2)
        elif mi == 12:
            final_out(1)
        elif mi == 14:
            final_out(2)

    EXTRA = {0: extra_h0, 1: extra_h1, 2: extra_h2, 3: extra_h3}

    # ---- main stream (deep lag early while v columns load, then shallow) ----
    for s in range(3, 4 * NT):
        h, mi = divmod(s, NT)
        items.append(dots_exp(h, mi))
        lag = 4 if s < 20 else 1
        while len(items) > lag:
            ot_flush(items.pop(0))
        EXTRA[h](mi)
    for it in items:
        ot_flush(it)

    # ---- tail: head 3 output rows live in otps[3]; stage + DMA per group ----
    for g in range(4):
        oos = oop.tile([P, 256], f32, tag="oo3", bufs=4)
        if g % 2 == 0:
            nc.vector.tensor_copy(oos[:], otps[3][:, g * 4 * D:(g + 1) * 4 * D])
        else:
            nc.scalar.copy(oos[:], otps[3][:, g * 4 * D:(g + 1) * 4 * D])
        oosv = oos.rearrange("p (s d) -> p s d", d=D)
        nc.sync.dma_start(oor[:, 4 * g:4 * g + 4, 3 * D:4 * D], oosv[:])

    for p in reversed(ctxs):
        p.__exit__(None, None, None)


def _get_prog():
    if "nc" not in _CACHE:
        _CACHE["nc"] = _build()
    return _CACHE["nc"]


def kernel(x, y, W_qkv, W_qkv_side):
    from concourse.bass_utils import run_bass_kernel_spmd

    nc = _get_prog()
    x = np.asarray(x, dtype=np.float32)
    y = np.asarray(y, dtype=np.float32)
    W_qkv = np.asarray(W_qkv, dtype=np.float32)
    W_qkv_side = np.asarray(W_qkv_side, dtype=np.float32)

    in_maps = []
    for c in range(NCORES):
        b, g = divmod(c, GROUPS)
        lo, hi = g * HD, (g + 1) * HD
        wx_c = np.ascontiguousarray(np.concatenate(
            [W_qkv[:, lo:hi], W_qkv[:, 2 * DIMX + lo:2 * DIMX + hi],
             W_qkv[:, DIMX + lo:DIMX + hi]], axis=1))
        wy_c = np.ascontiguousarray(np.concatenate(
            [W_qkv_side[:, DIMX + lo:DIMX + hi], W_qkv_side[:, lo:hi]], axis=1))
        in_maps.append({
            "x": np.ascontiguousarray(x[b]),
            "y": np.ascontiguousarray(y[b]),
            "wx": wx_c,
            "wy": wy_c,
        })

    _CACHE["in_maps_last"] = in_maps
    res = run_bass_kernel_spmd(nc, in_maps, core_ids=list(range(NCORES)))
    _CACHE["last_results"] = res

    v_full = np.empty((B, N, H * D), dtype=np.float32)
    o_full = np.empty((B, N, H * D), dtype=np.float32)
    for c in range(NCORES):
        b, g = divmod(c, GROUPS)
        v_full[b, :, g * HD:(g + 1) * HD] = res.results[c]["vout"]
        o_full[b, :, g * HD:(g + 1) * HD] = res.results[c]["oout"]
    return (v_full, o_full)
